# revision 15
# baseline (speedup 1.0000x reference)
"""Trainium2 Bass kernel for nn_ProjectLoss (bce + min-dist affinity loss).

Reference computes, per (b,h,w):
  loss        = -g*ln(p+EPS) - (1-g)*ln(|1-p-EPS|)
  min_dist    = min_{ij} [ gt_th * (grid[h,w,i,j]+1) * p ],   gt_th = g + (1-g)*BIG
  min_dist_inv= min_{ij} [ g * (grid[h,w,i,j]+1) * pm ],      pm    = p + (1-p)*BIG

Since gt_th, p, g, pm >= 0 and fp32 min is exact, the min over (i,j)
factors: min_dist = (gt_th*p) * (min_ij grid + 1).  The whole
[B,H,W,64,64] reduction collapses to a row-min of the raw grid followed
by a tiny elementwise epilogue (ulp-level reassociation differences,
far inside the 2e-2 tolerance).

Sharding: grid [64,64,64,64] split along H across 8 cores -> per-core
[8,64,64,64] viewed as [512,4096]; preds/gts sliced to the same 8 h-rows
and pre-transposed on host into the kernel's partition layout.

Per-core pipeline (stream is HBM-bound at ~385 GB/s, the per-NC cap):
  sync   : 9 HWDGE grid DMAs (2 MiB per row block, tapered tail so the
           last transfer is 128 KiB), then the two out flushes
  scalar : pg DMA + all activations (single-op forms: gt_th = BIG-(BIG-1)g)
  gpsimd : A = gt_th*p, B = g*pm, loss combine; mdi tail muls
  vector : one tensor_tensor_reduce per chunk - elementwise min of the
           chunk's two halves fused with the row-reduce and chained via
           the accum initial value, so DVE processes 2 elem/lane/cycle
           and needs no separate combine ops; md tail muls
"""

import sys

sys.path.insert(0, "/opt/trn_rl_repo")

import numpy as np
from contextlib import ExitStack

import concourse.bass as bass
from concourse import mybir
from concourse import dve_ops as _dve_ops
from concourse.bass_utils import run_bass_kernel_spmd
from concourse.dve_spec import (
    C1 as _C1,
    Spec as _Spec,
    Src0 as _Src0,
    Src1 as _Src1,
    _has_src1,
    lower as _dve_lower,
    minn as _minn,
)
from concourse.dve_uop import DveOpSpec as _DveOpSpec

EPS = 1e-08
BIG = 1000000.0
F32 = mybir.dt.float32
AF = mybir.ActivationFunctionType
ALU = mybir.AluOpType

N_CORES = 8
B, H, W = 2, 64, 64
HC = H // N_CORES          # h-rows per core = 8
ROWS = HC * W              # (h,w) pairs per core = 512
COLS = W * W               # (i,j) per (h,w) = 4096
RB = ROWS // 128           # row blocks of 128 partitions = 4

_NC_CACHE = {}

# Grid chunks (rowblock, col_off, width).  rb0-2 stream as whole 2 MiB
# blocks (DVE's fused reduce keeps up easily); rb3 tapers so the final
# reduce after the last HBM byte is tiny.
CHUNKS = [
    (0, 0, 4096), (1, 0, 4096), (2, 0, 4096),
    (3, 0, 1024), (3, 1024, 1024), (3, 2048, 1024),
    (3, 3072, 512), (3, 3584, 256), (3, 3840, 256),
]
# accum slot (parts column) each chunk's fused reduce writes; rb0-2
# write the final md_raw columns 8..10 directly, rb3's six chunks land
# in columns 0..5 and a final 6-wide stock min folds them into col 11.
# (Seeding an op's accum from the immediately preceding op's accum_out
# races on HW - the sequencer prefetches the scalar - so every op seeds
# from an immediate instead and the fold runs as a separate reduce.)
ACC = [8, 9, 10, 0, 1, 2, 3, 4, 5]

# Custom fused DVE op: out = min(in0, in1); accum_out = min(min_k out, s1).
# One instruction consumes two tensor streams (both SBUF read ports), so
# the grid min-reduce runs at 2 input elems/lane/cycle - 2x the rate of
# a stock tensor_reduce - and the s1 seed chains partials across chunks
# with no separate combine ops.  Registered per-NEFF via the custom-DVE
# table (04-custom-dve-api.md); no firmware change involved.
_MINR_NAME = "TT_MIN_REDUCE_ANT"


def _min_reduce_op():
    for op in _dve_ops.OPS:
        if op.name == _MINR_NAME:
            return op
    spec = _Spec(
        body=_minn(_Src0, _Src1),
        accum=_minn,
        accum_init=_C1,
        reference=lambda in0, in1, s0, s1, imm2: (
            np.minimum(in0, in1),
            np.minimum(
                np.minimum(in0, in1)
                .reshape(in0.shape[0], -1)
                .min(axis=-1, keepdims=True),
                np.asarray(s1, np.float32).reshape(-1, 1),
            ),
        ),
    )
    opcode = _dve_ops._CUSTOM_DVE_ROW_BASE + len(_dve_ops.OPS)
    assert opcode < 0x20
    _dve_ops._SUB_OPCODE_FOR_NAME[_MINR_NAME] = opcode
    sha = {}
    for ver in ("v3", "v4"):
        s = _DveOpSpec(name=_MINR_NAME, opcode=opcode,
                       uops=_dve_lower(spec, ver=ver), rd1_en=_has_src1(spec))
        sha[ver] = s.sha(ver)
    op = _dve_ops.DveOp(_MINR_NAME, spec, subdim=False, uops_sha=sha)
    _dve_ops.OPS.append(op)
    _dve_ops.CUSTOM_DVE_SPECS[_MINR_NAME] = spec
    return op


def _build():
    """Raw Bass program (no Tile): manual engines + semaphores."""
    # Skip the Bass-init all-engine barrier.  It protects the 0.0/1.0
    # const APs; our only read of them (Ln/Abs bias=0.0) happens on ACT
    # after the csem fence from GpSimd, whose program order already
    # places the const memsets first.  Saves ~6 us of boot wait.
    nc = bass.Bass("TRN2", target_bir_lowering=False, debug=False,
                   num_devices=N_CORES)
    grid = nc.declare_dram_parameter("grid", [ROWS, COLS], F32, isOutput=False)
    pg = nc.declare_dram_parameter("pg", [128, 16], F32, isOutput=False)
    out = nc.declare_dram_parameter("out", [128, 24], F32, isOutput=True)

    minr = _min_reduce_op()
    sb = lambda name, shape: nc.alloc_sbuf_tensor(name, shape, F32).ap()
    gt_tiles = [sb(f"gchunk{k}", [128, w]) for k, (_, _, w) in enumerate(CHUNKS)]
    scr = sb("scr", [128, 2048])     # fused-reduce elementwise dst (reused)
    pgt = sb("pgt", [128, 16])
    p = pgt[:, 0:8]
    g = pgt[:, 8:16]
    ot = sb("ot", [128, 24])
    parts = sb("parts", [128, 12])   # cols 0..4 rb3 chain, 8..11 md_raw
    md4 = sb("md4", [128, 4])
    gt_th = sb("gt_th", [128, 8])
    pm = sb("pm", [128, 8])
    ng = sb("ng", [128, 8])
    gm1 = sb("gm1", [128, 8])
    t1 = sb("t1", [128, 8])
    lnp = sb("lnp", [128, 8])
    t2 = sb("t2", [128, 8])
    ab = sb("ab", [128, 8])
    ln2 = sb("ln2", [128, 8])
    ta = sb("ta", [128, 8])          # A = gt_th * p
    tb = sb("tb", [128, 8])          # B = g * pm
    u = sb("u", [128, 8])
    v = sb("v", [128, 8])
    fence = sb("fence", [128, 1])

    with ExitStack() as ctx:
        block = ctx.enter_context(nc.Block(no_gpsimd_drain=True))
        gsem = [ctx.enter_context(nc.semaphore(f"gsem{k}"))
                for k in range(len(CHUNKS))]
        psem = ctx.enter_context(nc.semaphore("psem"))
        csem = ctx.enter_context(nc.semaphore("csem"))
        asem = ctx.enter_context(nc.semaphore("asem"))
        gseq = ctx.enter_context(nc.semaphore("gseq"))
        vseq = ctx.enter_context(nc.semaphore("vseq"))
        odone = ctx.enter_context(nc.semaphore("odone"))
        osem = ctx.enter_context(nc.semaphore("osem"))

        @block.sync
        def _(sync: bass.BassEngine):
            for k, (i, off, w) in enumerate(CHUNKS):
                sync.dma_start(
                    out=gt_tiles[k],
                    in_=grid[128 * i:128 * (i + 1), off:off + w],
                ).then_inc(gsem[k], 16)
            # loss columns are ready long before md/mdi -> flush early so
            # the final DMA is small and fully overlapped with teardown.
            sync.wait_ge(gseq, 2)
            sync.dma_start(out=out[:, 0:8], in_=ot[:, 0:8]).then_inc(osem, 16)
            sync.wait_ge(odone, 2)
            sync.dma_start(out=out[:, 8:24], in_=ot[:, 8:24]).then_inc(osem, 16)

        @block.scalar
        def _(act: bass.BassEngine):
            act.dma_start(out=pgt, in_=pg[:]).then_inc(psem, 16)
            act.wait_ge(csem, 1)   # const-0 AP written (gp program order)
            act.wait_ge(psem, 16)
            # single-op forms; Copy takes float imm bias/scale
            act.activation(gt_th, g, AF.Copy, bias=BIG,
                           scale=-(BIG - 1.0)).then_inc(asem)
            act.activation(pm, p, AF.Copy, bias=BIG,
                           scale=-(BIG - 1.0)).then_inc(asem)
            act.activation(ng, g, AF.Copy, scale=-1.0).then_inc(asem)
            act.activation(t1, p, AF.Copy, bias=EPS).then_inc(asem)
            act.activation(lnp, t1, AF.Ln).then_inc(asem)          # 5
            act.activation(gm1, g, AF.Copy, bias=-1.0).then_inc(asem)
            act.activation(t2, p, AF.Copy, bias=1.0 - EPS,
                           scale=-1.0).then_inc(asem)
            act.activation(ab, t2, AF.Abs).then_inc(asem)
            act.activation(ln2, ab, AF.Ln).then_inc(asem)          # 9

        @block.gpsimd
        def _(gp: bass.BassEngine):
            gp.memset(fence, 0.0).then_inc(csem)
            gp.wait_ge(asem, 2)
            gp.tensor_mul(ta, gt_th, p)
            gp.tensor_mul(tb, g, pm).then_inc(gseq)         # 1: A,B ready
            gp.wait_ge(asem, 5)
            gp.tensor_mul(u, ng, lnp)
            gp.wait_ge(asem, 9)
            gp.tensor_mul(v, gm1, ln2)
            gp.tensor_add(ot[:, 0:8], u, v).then_inc(gseq)  # 2: loss ready
            gp.wait_ge(vseq, 3)   # md4 ready
            gp.tensor_mul(ot[:, 16:20], tb[:, 0:4], md4)
            gp.tensor_mul(ot[:, 20:24], tb[:, 4:8], md4).then_inc(odone)

        @block.vector
        def _(vec: bass.BassEngine):
            # Tight read-after-write pairs on DVE need a wait_ge on the
            # producer's completion semaphore in between: the sequencer
            # prefetches the next instruction's input streams while the
            # current one drains, so an unfenced consumer reads stale
            # bytes (accum_out writes land at drain, not issue).
            for k, (i, off, w) in enumerate(CHUNKS):
                h = w // 2
                vec.wait_ge(gsem[k], 16)
                ins = vec._custom_dve(
                    minr,
                    out=scr[:, 0:h],
                    in0=gt_tiles[k][:, 0:h],
                    in1=gt_tiles[k][:, h:w],
                    s1=3.0e38,
                    accum_out=parts[:, ACC[k]:ACC[k] + 1],
                )
            ins.then_inc(vseq)     # last chunk's accum write fence
            vec.wait_ge(vseq, 1)
            vec.tensor_reduce(parts[:, 11:12], parts[:, 0:6],
                              axis=mybir.AxisListType.X,
                              op=ALU.min).then_inc(vseq)
            vec.wait_ge(vseq, 2)
            vec.tensor_scalar_add(md4, parts[:, 8:12], 1.0).then_inc(vseq)
            vec.wait_ge(vseq, 3)   # md4 write fence
            vec.wait_ge(gseq, 1)   # A ready
            vec.tensor_mul(ot[:, 8:12], ta[:, 0:4], md4)
            vec.tensor_mul(ot[:, 12:16], ta[:, 4:8], md4).then_inc(odone)

    # Raw Bass skips Bacc's codegen_inst_isa_subclasses pass; without it
    # the custom-DVE instruction ships empty .instr bytes and the NEFF
    # compiler rejects it with "ISA wrong length".
    mybir.codegen_inst_isa_subclasses(nc)
    return nc


def get_nc():
    if "nc" not in _NC_CACHE:
        _NC_CACHE["nc"] = _build()
    return _NC_CACHE["nc"]


def make_in_maps(preds, gts, grid):
    preds = np.ascontiguousarray(np.asarray(preds, dtype=np.float32))
    gts = np.ascontiguousarray(np.asarray(gts, dtype=np.float32))
    grid = np.ascontiguousarray(np.asarray(grid, dtype=np.float32))
    in_maps = []
    for c in range(N_CORES):
        gslice = np.ascontiguousarray(
            grid[HC * c:HC * (c + 1)].reshape(ROWS, COLS))
        pf = preds[:, HC * c:HC * (c + 1), :].reshape(B, ROWS)
        gf = gts[:, HC * c:HC * (c + 1), :].reshape(B, ROWS)
        pg = np.empty((128, 16), np.float32)
        for b in range(B):
            for t in range(RB):
                pg[:, 4 * b + t] = pf[b, 128 * t:128 * (t + 1)]
                pg[:, 8 + 4 * b + t] = gf[b, 128 * t:128 * (t + 1)]
        in_maps.append({"grid": gslice, "pg": pg})
    return in_maps


def unshard(results):
    loss = np.empty((B, H, W), np.float32)
    md = np.empty((B, H, W), np.float32)
    mdi = np.empty((B, H, W), np.float32)
    for c in range(N_CORES):
        o = results[c]["out"]  # [128, 24]
        for b in range(B):
            for t in range(RB):
                rows = slice(128 * t, 128 * (t + 1))
                loss[b, HC * c:HC * (c + 1)].reshape(ROWS)[rows] = o[:, 4 * b + t]
                md[b, HC * c:HC * (c + 1)].reshape(ROWS)[rows] = o[:, 8 + 4 * b + t]
                mdi[b, HC * c:HC * (c + 1)].reshape(ROWS)[rows] = o[:, 16 + 4 * b + t]
    return loss, md, mdi


def run(preds, gts, grid_dist_tensor, trace=False, **trace_kwargs):
    nc = get_nc()
    in_maps = make_in_maps(preds, gts, grid_dist_tensor)
    res = run_bass_kernel_spmd(nc, in_maps, list(range(N_CORES)), trace=trace,
                               **trace_kwargs)
    return unshard(res.results), res


def kernel(**inputs):
    (loss, md, mdi), _ = run(inputs["preds"], inputs["gts"],
                             inputs["grid_dist_tensor"])
    return loss, md, mdi
